# revision 2
# baseline (speedup 1.0000x reference)
# Trainium2 Bass kernel for nn_ClassBlock (mamba + EinFFT class-token block).
#
# The block only transforms x[:, :1] (the CLS token); x[:, 1:] passes through
# untouched.  Per batch row b (sequence length 1, h0 = 0) the math collapses to:
#   ln1  = layernorm(cls)
#   xz   = ln1 @ in_proj_w.T            -> xin, z (3072 each)
#   xc   = silu(xin * conv_w[:,3] + conv_b)
#   dbl  = xc @ x_proj_w.T              -> dt0(96), B(64), C(64)
#   dt   = softplus(dt0 @ dt_proj_w.T + dt_proj_b)
#   s    = dot(B, C)                    (scan with L=1, h0=0)
#   y    = xc * (dt*s + D) * silu(z)
#   mam  = y @ out_proj_w.T
#   cls2 = cls + mam
#   ln2  = layernorm(cls2)
#   einfft: FFT4 over the 4 blocks of 384 -> complex block matmuls (relu,
#           softshrink) -> IFFT4 real part; out = cls2 + einfft(ln2)
#
# Sharding: pure data parallel, 8 batch rows per core (64 total / 8 cores).
# On-chip layout: transposed activations [features(partitions), batch(free=8)].
# All weights are pre-transposed/packed on host so every DMA is a clean
# contiguous slab, and are used as the stationary matmul operand (lhsT).
# FFT4/IFFT4 are explicit adds/subs; both 1/sqrt(4) ortho factors are folded
# into the einfft weights/biases host-side (softshrink lambda rescaled to l/2).

import numpy as np
from contextlib import ExitStack

import concourse.bass as bass
import concourse.mybir as mybir
import concourse.tile as tile
from concourse.bass_utils import run_bass_kernel_spmd

F32 = mybir.dt.float32
BF16 = mybir.dt.bfloat16
AF = mybir.ActivationFunctionType
ALU = mybir.AluOpType

NCORES = 8
SIM_COMPAT = False    # compose Silu/Softplus from sim-supported primitives
R = 8                 # batch rows per core
DIM = 1536
NCH = DIM // 128      # 12 feature chunks
DI = 3072
DICH = DI // 128      # 24 d_inner chunks
DT_RANK = 96
EPS = 1e-5
LAM2 = 0.005          # softshrink lambda (0.01) folded by the 1/2 ifft factor

# smallvec column layout (per-feature vectors packed as [128, col])
SV_N1G = 0
SV_N1B = 12
SV_CW3 = 24
SV_CB = 48
SV_DTB = 72
SV_D = 96
SV_N2G = 120
SV_N2B = 132
SV_CB1 = 144          # cb1: ri*12 + b*3 + c
SV_SS1 = 168          # 0.5*cb2 - LAM2
SV_SS2 = 192          # -0.5*cb2 - LAM2
SV_TOT = 216


def _silu_act(nc, tmp, out_ap, ps_ap, scale=None, bias=None):
    """out = silu(ps*scale + bias); scale/bias optional [P,1] APs."""
    kw = {}
    if scale is not None:
        kw["scale"] = scale
    if bias is not None:
        kw["bias"] = bias
    if not SIM_COMPAT:
        nc.scalar.activation(out_ap, ps_ap, AF.Silu, **kw)
        return
    if scale is not None or bias is not None:
        v = tmp.tile([128, R], F32, tag="tmp", name="silu_v")
        nc.vector.tensor_scalar(v[:], ps_ap, scale if scale is not None else 1.0,
                                bias if bias is not None else 0.0,
                                ALU.mult, ALU.add)
        v_ap = v[:]
    else:
        v_ap = ps_ap
    sg = tmp.tile([128, R], F32, tag="tmp", name="silu_sg")
    nc.scalar.activation(sg[:], v_ap, AF.Sigmoid)
    nc.vector.tensor_mul(out_ap, v_ap, sg[:])


def _softplus_act(nc, pools, out_ap, ps_ap, bias):
    # softplus(x) = ln(1 + exp(x)); walrus act tables have no native softplus.
    tmp = pools["tmp"]
    e = tmp.tile([128, R], F32, tag="tmp", name="sp_e")
    nc.scalar.activation(e[:], ps_ap, AF.Exp, bias=bias)
    nc.scalar.activation(out_ap, e[:], AF.Ln, bias=pools["ones128"][:, 0:1])


def _layernorm_t(nc, pools, x_tiles, gcol, bcol, out_tag, out_dt=F32):
    """LayerNorm over features for transposed tiles x_tiles (12 x [128,8])."""
    psmall, tmp, sv, ones128, ones1, outp = (
        pools["psmall"], pools["tmp"], pools["sv"], pools["ones128"],
        pools["ones1"], pools["acts"],
    )
    eps_t = pools["eps"]
    sum_ps = psmall.tile([1, R], F32, tag="psmall")
    ssq_ps = psmall.tile([1, R], F32, tag="psmall")
    for c in range(NCH):
        sq = tmp.tile([128, R], F32, tag="tmp")
        nc.scalar.activation(sq[:], x_tiles[c][:], AF.Square)
        nc.tensor.matmul(sum_ps[:], ones128[:], x_tiles[c][:],
                         start=(c == 0), stop=(c == NCH - 1))
        nc.tensor.matmul(ssq_ps[:], ones128[:], sq[:],
                         start=(c == 0), stop=(c == NCH - 1))
    m_sb = tmp.tile([1, R], F32, tag="s1")
    nc.scalar.activation(m_sb[:], sum_ps[:], AF.Copy, scale=1.0 / DIM)
    ms_sb = tmp.tile([1, R], F32, tag="s1")
    nc.scalar.activation(ms_sb[:], ssq_ps[:], AF.Copy, scale=1.0 / DIM)
    mm_sb = tmp.tile([1, R], F32, tag="s1")
    nc.vector.tensor_mul(mm_sb[:], m_sb[:], m_sb[:])
    var_sb = tmp.tile([1, R], F32, tag="s1")
    nc.vector.tensor_sub(var_sb[:], ms_sb[:], mm_sb[:])
    # rstd = (var+eps)^-0.5 = exp(-0.5*ln(var+eps)); sqrt is not in the
    # silu/ln/exp ACT table sets this walrus build can load together.
    lnv_sb = tmp.tile([1, R], F32, tag="s1")
    nc.scalar.activation(lnv_sb[:], var_sb[:], AF.Ln, bias=eps_t[:])
    rstd_sb = tmp.tile([1, R], F32, tag="s1")
    nc.scalar.activation(rstd_sb[:], lnv_sb[:], AF.Exp, scale=-0.5)
    mr_sb = tmp.tile([1, R], F32, tag="s1")
    nc.vector.tensor_mul(mr_sb[:], m_sb[:], rstd_sb[:])
    bcsrc = tmp.tile([1, 2 * R], F32, tag="s2")
    nc.vector.tensor_copy(bcsrc[:, 0:R], rstd_sb[:])
    nc.vector.tensor_copy(bcsrc[:, R:2 * R], mr_sb[:])
    bc_ps = psmall.tile([128, 2 * R], F32, tag="psmall")
    nc.tensor.matmul(bc_ps[:], ones1[:], bcsrc[:], start=True, stop=True)
    bc_sb = tmp.tile([128, 2 * R], F32, tag="s3")
    nc.scalar.activation(bc_sb[:], bc_ps[:], AF.Copy)

    outs = []
    for c in range(NCH):
        t = tmp.tile([128, R], F32, tag="tmp")
        nc.vector.tensor_mul(t[:], x_tiles[c][:], bc_sb[:, 0:R])
        t2 = tmp.tile([128, R], F32, tag="tmp")
        nc.vector.tensor_sub(t2[:], t[:], bc_sb[:, R:2 * R])
        o = outp.tile([128, R], out_dt, tag=out_tag)
        nc.vector.tensor_scalar(o[:], t2[:], sv[:, gcol + c:gcol + c + 1],
                                sv[:, bcol + c:bcol + c + 1], ALU.mult, ALU.add)
        outs.append(o)
    return outs


class _SplitDrainTC(tile.TileContext):
    """TileContext whose kernel-tail drain carries at most one sem wait.

    The neuronxcc walrus build used under axon rejects CTRL instructions
    with several sync waits ("Too many sync wait commands"), so the excess
    waits are peeled onto extra single-wait drains.
    """

    def _drain_and_barrier(self, tick_clock, wait_clock):
        from concourse.vector_clock import ScopedClock

        drain_inst = self.nc.sync.drain()
        wait_clock.add_sem_waits(
            drain_inst.ins, ScopedClock({None: tick_clock.global_clock})
        )
        si = drain_inst.ins.sync_info
        if si is not None and len(si.on_wait) > 1:
            waits = list(si.on_wait)
            drain_inst.ins.sync_info = mybir.SyncInfo(
                on_wait=[waits[0]], on_update=list(si.on_update)
            )
            for w in waits[1:]:
                d2 = self.nc.sync.drain()
                d2.ins.sync_info = mybir.SyncInfo(on_wait=[w], on_update=[])

        self.nc.all_engine_barrier()
        assert self.sems is not None
        popped = self.nc._tile_sem_poison_stack.pop()
        assert popped is self._sem_poison
        self.nc.clear_and_free_semaphores(list(self.sems.allocated().values()))
        self.nc.all_engine_barrier()


def _split_waits(nc, maxw=1):
    """Walrus (neuronxcc) allows very few sync waits per ISA instruction.

    Peel excess sem waits off every instruction onto same-engine NoOps
    inserted immediately before it -- semantically identical: the engine
    sequencer blocks on the NoOp's wait, then on the instruction's own.
    """
    for f in nc.m.functions:
        for blk in f.blocks:
            insts = list(blk.instructions)
            out = []
            changed = False
            for inst in insts:
                si = inst.sync_info
                if si is not None and len(si.on_wait) > maxw:
                    waits = list(si.on_wait)
                    for j, w in enumerate(waits[maxw:]):
                        nop = mybir.InstNoOp(
                            name=f"{inst.name}.wsp{j}", engine=inst.engine,
                            ins=[], outs=[],
                            sync_info=mybir.SyncInfo(on_wait=[w], on_update=[]),
                        )
                        out.append(nop)
                    inst.sync_info = mybir.SyncInfo(
                        on_wait=waits[:maxw], on_update=list(si.on_update)
                    )
                    changed = True
                out.append(inst)
            if changed:
                blk.instructions = out


def build_bass(wdt=F32, krep=1):
    nc = bass.Bass("TRN2")
    clsT_h = nc.declare_dram_parameter("clsT", [128, R * NCH], F32, isOutput=False)
    sv_h = nc.declare_dram_parameter("sv", [128, SV_TOT], F32, isOutput=False)
    w1_h = nc.declare_dram_parameter("w1", [12, 12, 128, 512], wdt, isOutput=False)
    xpw_h = nc.declare_dram_parameter("xpw", [24, 128, 384], wdt, isOutput=False)
    dtw_h = nc.declare_dram_parameter("dtw", [96, 3072], wdt, isOutput=False)
    w2_h = nc.declare_dram_parameter("w2", [6, 24, 128, 256], wdt, isOutput=False)
    fw_h = nc.declare_dram_parameter("fw", [2, 2, 4, 3, 128, 384], wdt, isOutput=False)
    out_h = nc.declare_dram_parameter("outT", [128, R * NCH], F32, isOutput=True)

    with _SplitDrainTC(nc) as tc:
        if krep == 1:
            with ExitStack() as ctx:
                _body(ctx, tc, nc, wdt, clsT_h, sv_h, w1_h, xpw_h, dtw_h,
                      w2_h, fw_h, out_h[:])
        else:
            # benchmarking variant: hardware loop repeats the whole body
            # (fresh weight streaming each iteration, same output slab) so
            # the wall-clock delta (T(krep)-T(1))/(krep-1) isolates the
            # steady-state per-iteration device time at constant compile cost
            with tc.For_i(0, krep):
                with ExitStack() as ctx:
                    _body(ctx, tc, nc, wdt, clsT_h, sv_h, w1_h, xpw_h, dtw_h,
                          w2_h, fw_h, out_h[:])
    if not SIM_COMPAT:
        # serialization-level workaround for walrus; CoreSim can't replay it
        _split_waits(nc)
    return nc


def _body(ctx, tc, nc, wdt, clsT_h, sv_h, w1_h, xpw_h, dtw_h, w2_h, fw_h, out_ap):
    const = ctx.enter_context(tc.tile_pool(name="const", bufs=1))
    wbig = ctx.enter_context(tc.tile_pool(name="wbig", bufs=3))
    fwp = ctx.enter_context(tc.tile_pool(name="fwp", bufs=2))
    tmp = ctx.enter_context(tc.tile_pool(name="tmp", bufs=8))
    acts = ctx.enter_context(tc.tile_pool(name="acts", bufs=24))
    pps = ctx.enter_context(tc.tile_pool(name="pps", bufs=6, space="PSUM"))
    psmall = ctx.enter_context(tc.tile_pool(name="psmall", bufs=2, space="PSUM"))

    # constants / small inputs
    clsT = const.tile([128, R * NCH], F32)
    nc.sync.dma_start(clsT[:], clsT_h[:])
    sv = const.tile([128, SV_TOT], F32)
    nc.sync.dma_start(sv[:], sv_h[:])
    ones128 = const.tile([128, 1], F32)
    nc.vector.memset(ones128[:], 1.0)
    ones1 = const.tile([1, 128], F32)
    nc.vector.memset(ones1[:], 1.0)
    eps_t = const.tile([1, 1], F32)
    nc.vector.memset(eps_t[:], EPS)
    xp_sb = const.tile([128, 24 * 384], wdt)
    nc.sync.dma_start(xp_sb[:].rearrange("p (k j) -> p k j", k=24),
                      xpw_h[:].rearrange("k p j -> p k j"))
    dtw_sb = const.tile([96, 3072], wdt)
    nc.sync.dma_start(dtw_sb[:], dtw_h[:])

    pools = {"psmall": psmall, "tmp": tmp, "sv": sv, "ones128": ones128,
             "ones1": ones1, "acts": acts, "eps": eps_t}

    adt = wdt
    cls_tiles = [clsT[:, c * R:(c + 1) * R] for c in range(NCH)]
    ln1 = _layernorm_t(nc, pools, cls_tiles, SV_N1G, SV_N1B, "ln1", out_dt=adt)

    # ---- in_proj: xz[f, b] for f in 6144; xin -> xc (silu conv), z -> silu(z)
    xc = [None] * DICH
    sz = [None] * DICH
    for g in range(12):
        w1sb = wbig.tile([128, 6144], wdt, tag="wbig")
        nc.sync.dma_start(w1sb[:].rearrange("p (k j) -> p k j", k=12),
                          w1_h[g].rearrange("k p j -> p k j"))
        for m in range(4):
            ps = pps.tile([128, R], F32, tag="ps8")
            for k in range(12):
                off = k * 512 + m * 128
                nc.tensor.matmul(ps[:], w1sb[:, off:off + 128], ln1[k][:],
                                 start=(k == 0), stop=(k == 11))
            gm = g * 4 + m
            if gm < DICH:
                t = acts.tile([128, R], adt, tag="xc")
                _silu_act(nc, tmp, t[:], ps[:],
                          scale=sv[:, SV_CW3 + gm:SV_CW3 + gm + 1],
                          bias=sv[:, SV_CB + gm:SV_CB + gm + 1])
                xc[gm] = t
            else:
                t = acts.tile([128, R], F32, tag="sz")
                _silu_act(nc, tmp, t[:], ps[:])
                sz[gm - DICH] = t

    # ---- x_proj -> dt0 [96], B [64], C [64]; s = dot(B, C)
    ps_dt = pps.tile([128, R], F32, tag="ps8")
    ps_B = pps.tile([128, R], F32, tag="ps8")
    ps_C = pps.tile([128, R], F32, tag="ps8")
    for k in range(24):
        base = k * 384
        nc.tensor.matmul(ps_dt[:], xp_sb[:, base:base + 128], xc[k][:],
                         start=(k == 0), stop=(k == 23))
        nc.tensor.matmul(ps_B[:], xp_sb[:, base + 128:base + 256], xc[k][:],
                         start=(k == 0), stop=(k == 23))
        nc.tensor.matmul(ps_C[:], xp_sb[:, base + 256:base + 384], xc[k][:],
                         start=(k == 0), stop=(k == 23))
    dtm = const.tile([96, R], adt)
    nc.scalar.activation(dtm[:], ps_dt[0:96, :], AF.Copy)
    bsb = tmp.tile([128, R], F32, tag="tmp")
    nc.scalar.activation(bsb[0:64, :], ps_B[0:64, :], AF.Copy)
    bcm = tmp.tile([128, R], F32, tag="tmp")
    nc.vector.tensor_mul(bcm[0:64, :], bsb[0:64, :], ps_C[0:64, :])
    s_ps = psmall.tile([1, R], F32, tag="psmall")
    nc.tensor.matmul(s_ps[:], ones128[0:64, :], bcm[0:64, :], start=True, stop=True)
    s_sb = tmp.tile([1, R], F32, tag="s1")
    nc.scalar.activation(s_sb[:], s_ps[:], AF.Copy)
    sbc_ps = psmall.tile([128, R], F32, tag="psmall")
    nc.tensor.matmul(sbc_ps[:], ones1[:], s_sb[:], start=True, stop=True)
    s_bc = const.tile([128, R], F32)
    nc.scalar.activation(s_bc[:], sbc_ps[:], AF.Copy)

    # ---- dt = softplus(dt0 @ dtw + b); y = xc * (dt*s + D) * silu(z)
    y = [None] * DICH
    for m in range(DICH):
        ps = pps.tile([128, R], F32, tag="ps8")
        nc.tensor.matmul(ps[:], dtw_sb[:, m * 128:(m + 1) * 128], dtm[:],
                         start=True, stop=True)
        dt_t = tmp.tile([128, R], F32, tag="tmp")
        _softplus_act(nc, pools, dt_t[:], ps[:],
                      bias=sv[:, SV_DTB + m:SV_DTB + m + 1])
        dts = tmp.tile([128, R], F32, tag="tmp")
        nc.vector.tensor_mul(dts[:], dt_t[:], s_bc[:])
        xsz = tmp.tile([128, R], F32, tag="tmp")
        nc.vector.tensor_mul(xsz[:], xc[m][:], sz[m][:])
        yt = acts.tile([128, R], adt, tag="y")
        nc.vector.scalar_tensor_tensor(yt[:], dts[:], sv[:, SV_D + m:SV_D + m + 1],
                                       xsz[:], ALU.add, ALU.mult)
        y[m] = yt

    # ---- out_proj + residual
    cls2 = [None] * NCH
    for g in range(6):
        w2sb = wbig.tile([128, 6144], wdt, tag="wbig")
        nc.sync.dma_start(w2sb[:].rearrange("p (k j) -> p k j", k=24),
                          w2_h[g].rearrange("k p j -> p k j"))
        for m in range(2):
            ps = pps.tile([128, R], F32, tag="ps8")
            for k in range(24):
                off = k * 256 + m * 128
                nc.tensor.matmul(ps[:], w2sb[:, off:off + 128], y[k][:],
                                 start=(k == 0), stop=(k == 23))
            ci = g * 2 + m
            t = acts.tile([128, R], F32, tag="cls2")
            nc.vector.tensor_add(t[:], ps[:], clsT[:, ci * R:(ci + 1) * R])
            cls2[ci] = t

    ln2 = _layernorm_t(nc, pools, cls2, SV_N2G, SV_N2B, "ln2")  # f32; fft combos cast

    # ---- FFT4 across blocks (x_j = ln2 block j), unscaled (1/2 folded into fw)
    xr0 = [None] * 3
    xr1 = [None] * 3
    xr2 = [None] * 3
    t31 = [None] * 3  # x3 - x1 =  xi1 (unscaled)
    t13 = [None] * 3  # x1 - x3 = -xi1
    for c in range(3):
        x0, x1, x2, x3 = ln2[c], ln2[3 + c], ln2[6 + c], ln2[9 + c]
        p = tmp.tile([128, R], F32, tag="tmp")
        nc.vector.tensor_add(p[:], x0[:], x2[:])
        q = tmp.tile([128, R], F32, tag="tmp")
        nc.vector.tensor_add(q[:], x1[:], x3[:])
        xr0[c] = acts.tile([128, R], adt, tag="fft", name=f"xr0_{c}")
        nc.vector.tensor_add(xr0[c][:], p[:], q[:])
        xr2[c] = acts.tile([128, R], adt, tag="fft", name=f"xr2_{c}")
        nc.vector.tensor_sub(xr2[c][:], p[:], q[:])
        xr1[c] = acts.tile([128, R], adt, tag="fft", name=f"xr1_{c}")
        nc.vector.tensor_sub(xr1[c][:], x0[:], x2[:])
        t31[c] = acts.tile([128, R], adt, tag="fft", name=f"t31_{c}")
        nc.vector.tensor_sub(t31[c][:], x3[:], x1[:])
        t13[c] = acts.tile([128, R], adt, tag="fft", name=f"t13_{c}")
        nc.vector.tensor_sub(t13[c][:], x1[:], x3[:])

    # ---- einfft layer 1: r1 = relu(xr@W0 - xi@W1 + cb1r); i1 = relu(xr@W1 + xi@W0 + cb1i)
    def fw_load(l, w):
        t = fwp.tile([128, 4608], wdt, tag="fw")
        nc.sync.dma_start(t[:].rearrange("p (b kc m) -> p b kc m", b=4, kc=3),
                          fw_h[l, w].rearrange("b kc p m -> p b kc m"))
        return t

    fw10 = fw_load(0, 0)
    fw11 = fw_load(0, 1)

    xr_of = [xr0, xr1, xr2, xr1]
    xi_of = [None, t31, None, t13]
    nxi_of = [None, t13, None, t31]
    r1t = {}
    i1t = {}
    i1nt = {}
    for b in range(4):
        for mc in range(3):
            ps_r = pps.tile([128, R], F32, tag="ps8")
            ps_i = pps.tile([128, R], F32, tag="ps8")
            has_xi = xi_of[b] is not None
            for kc in range(3):
                c0 = b * 1152 + kc * 384 + mc * 128
                last = (kc == 2) and not has_xi
                nc.tensor.matmul(ps_r[:], fw10[:, c0:c0 + 128], xr_of[b][kc][:],
                                 start=(kc == 0), stop=last)
                nc.tensor.matmul(ps_i[:], fw11[:, c0:c0 + 128], xr_of[b][kc][:],
                                 start=(kc == 0), stop=last)
            if has_xi:
                for kc in range(3):
                    c0 = b * 1152 + kc * 384 + mc * 128
                    nc.tensor.matmul(ps_r[:], fw11[:, c0:c0 + 128],
                                     nxi_of[b][kc][:], start=False, stop=(kc == 2))
                    nc.tensor.matmul(ps_i[:], fw10[:, c0:c0 + 128],
                                     xi_of[b][kc][:], start=False, stop=(kc == 2))
            cr = SV_CB1 + b * 3 + mc
            ci_ = SV_CB1 + 12 + b * 3 + mc
            rt = acts.tile([128, R], adt, tag="r1")
            nc.scalar.activation(rt[:], ps_r[:], AF.Relu, bias=sv[:, cr:cr + 1])
            it = acts.tile([128, R], adt, tag="i1")
            nc.scalar.activation(it[:], ps_i[:], AF.Relu, bias=sv[:, ci_:ci_ + 1])
            intile = acts.tile([128, R], adt, tag="i1n")
            nc.vector.tensor_scalar_mul(intile[:], it[:], -1.0)
            r1t[b, mc] = rt
            i1t[b, mc] = it
            i1nt[b, mc] = intile

    # ---- einfft layer 2 + softshrink (only blocks 1,3 need the imag output)
    fw20 = fw_load(1, 0)
    fw21 = fw_load(1, 1)

    Rt = {}
    It = {}
    for b in range(4):
        need_i = b in (1, 3)
        for mc in range(3):
            ps_r = pps.tile([128, R], F32, tag="ps8")
            ps_i = pps.tile([128, R], F32, tag="ps8", name=f"psi2_{b}_{mc}") if need_i else None
            for kc in range(3):
                c0 = b * 1152 + kc * 384 + mc * 128
                nc.tensor.matmul(ps_r[:], fw20[:, c0:c0 + 128], r1t[b, kc][:],
                                 start=(kc == 0), stop=False)
                nc.tensor.matmul(ps_r[:], fw21[:, c0:c0 + 128], i1nt[b, kc][:],
                                 start=False, stop=(kc == 2))
                if need_i:
                    nc.tensor.matmul(ps_i[:], fw21[:, c0:c0 + 128], r1t[b, kc][:],
                                     start=(kc == 0), stop=False)
                    nc.tensor.matmul(ps_i[:], fw20[:, c0:c0 + 128], i1t[b, kc][:],
                                     start=False, stop=(kc == 2))
            plist = [(0, ps_r)] + ([(1, ps_i)] if need_i else [])
            for ri, ps in plist:
                c1 = SV_SS1 + ri * 12 + b * 3 + mc
                c2 = SV_SS2 + ri * 12 + b * 3 + mc
                a1 = tmp.tile([128, R], F32, tag="tmp")
                nc.scalar.activation(a1[:], ps[:], AF.Relu, bias=sv[:, c1:c1 + 1])
                a2 = tmp.tile([128, R], F32, tag="tmp")
                nc.scalar.activation(a2[:], ps[:], AF.Relu, bias=sv[:, c2:c2 + 1],
                                     scale=-1.0)
                o = acts.tile([128, R], F32, tag="RI")
                nc.vector.tensor_sub(o[:], a1[:], a2[:])
                if ri == 0:
                    Rt[b, mc] = o
                else:
                    It[b, mc] = o

    # ---- IFFT4 (real part, unscaled) + final residual; write [128, 96] out
    out_sb = const.tile([128, R * NCH], F32)
    for mc in range(3):
        R0, R1, R2, R3 = Rt[0, mc], Rt[1, mc], Rt[2, mc], Rt[3, mc]
        I1, I3 = It[1, mc], It[3, mc]
        a = tmp.tile([128, R], F32, tag="tmp")
        nc.vector.tensor_add(a[:], R0[:], R2[:])
        b2 = tmp.tile([128, R], F32, tag="tmp")
        nc.vector.tensor_add(b2[:], R1[:], R3[:])
        cc = tmp.tile([128, R], F32, tag="tmp")
        nc.vector.tensor_sub(cc[:], R0[:], R2[:])
        d2 = tmp.tile([128, R], F32, tag="tmp")
        nc.vector.tensor_sub(d2[:], I1[:], I3[:])
        combos = [(a, b2, ALU.add), (cc, d2, ALU.subtract),
                  (a, b2, ALU.subtract), (cc, d2, ALU.add)]
        for j, (u, v, op) in enumerate(combos):
            ch = 3 * j + mc
            t = tmp.tile([128, R], F32, tag="tmp")
            nc.vector.tensor_tensor(t[:], u[:], v[:], op)
            nc.vector.tensor_add(out_sb[:, ch * R:(ch + 1) * R], t[:],
                                 cls2[ch][:])
    nc.sync.dma_start(out_ap, out_sb[:])


# ---------------------------------------------------------------------------
# Host side
# ---------------------------------------------------------------------------

_NC_CACHE = {}
LAST_RES = None
TRACE = False
WDT = BF16  # weight dtype knob (bf16 halves the dominant weight-stream DMA)


def _np_wdt(wdt):
    if wdt == F32:
        return np.float32
    import ml_dtypes
    return ml_dtypes.bfloat16


def _get_nc(wdt):
    if wdt not in _NC_CACHE:
        _NC_CACHE[wdt] = build_bass(wdt)
    return _NC_CACHE[wdt]


def _chunkcols(v):
    """[C*128] feature vector -> [128, C] (feature f=128c+p at [p, c])."""
    v = np.asarray(v, np.float32)
    C = v.shape[0] // 128
    return v.reshape(C, 128).T


def host_prep(inputs, wdt=None):
    """Build the shared (per-core identical) device input arrays."""
    wdt = wdt or WDT
    nw = _np_wdt(wdt)
    g = lambda k: np.asarray(inputs[k], np.float32)

    A = g("in_proj_w")                       # [6144, 1536]
    w1 = np.ascontiguousarray(
        A.reshape(12, 512, 12, 128).transpose(0, 2, 3, 1)).astype(nw)

    PT = g("x_proj_w").T                     # [3072, 224]
    PTpad = np.zeros((3072, 384), np.float32)
    PTpad[:, 0:96] = PT[:, 0:96]
    PTpad[:, 128:192] = PT[:, 96:160]
    PTpad[:, 256:320] = PT[:, 160:224]
    xpw = np.ascontiguousarray(PTpad.reshape(24, 128, 384)).astype(nw)

    dtw = np.ascontiguousarray(g("dt_proj_w").T).astype(nw)     # [96, 3072]

    O = g("out_proj_w")                      # [1536, 3072]
    w2 = np.ascontiguousarray(
        O.reshape(6, 256, 24, 128).transpose(0, 2, 3, 1)).astype(nw)

    fw = np.stack([0.5 * g("cw1"), 0.5 * g("cw2")])  # [2, 2, 4, 384, 384]
    fw = np.ascontiguousarray(fw.reshape(2, 2, 4, 3, 128, 384)).astype(nw)

    sv = np.zeros((128, SV_TOT), np.float32)
    sv[:, SV_N1G:SV_N1G + 12] = _chunkcols(g("norm1_g"))
    sv[:, SV_N1B:SV_N1B + 12] = _chunkcols(g("norm1_b"))
    sv[:, SV_CW3:SV_CW3 + 24] = _chunkcols(g("conv_w")[:, 3])
    sv[:, SV_CB:SV_CB + 24] = _chunkcols(g("conv_b"))
    sv[:, SV_DTB:SV_DTB + 24] = _chunkcols(g("dt_proj_b"))
    sv[:, SV_D:SV_D + 24] = _chunkcols(g("D"))
    sv[:, SV_N2G:SV_N2G + 12] = _chunkcols(g("norm2_g"))
    sv[:, SV_N2B:SV_N2B + 12] = _chunkcols(g("norm2_b"))
    cb1 = g("cb1")
    cb2 = g("cb2")
    for ri in range(2):
        for b in range(4):
            c0 = SV_CB1 + ri * 12 + b * 3
            sv[:, c0:c0 + 3] = _chunkcols(cb1[ri, b])
            c0 = SV_SS1 + ri * 12 + b * 3
            sv[:, c0:c0 + 3] = _chunkcols(0.5 * cb2[ri, b] - LAM2)
            c0 = SV_SS2 + ri * 12 + b * 3
            sv[:, c0:c0 + 3] = _chunkcols(-0.5 * cb2[ri, b] - LAM2)

    return {"sv": sv, "w1": w1, "xpw": xpw, "dtw": dtw, "w2": w2, "fw": fw}


def make_clsT(cls, r):
    """cls [64, 1536] -> core r's [128, 96] transposed tile."""
    rr = cls[r * R:(r + 1) * R]              # [8, 1536]
    return np.ascontiguousarray(
        rr.T.reshape(NCH, 128, R).transpose(1, 0, 2).reshape(128, R * NCH))


def decode_out(o):
    """[128, 96] device output -> [8, 1536] cls rows."""
    o = np.asarray(o, np.float32)
    return o.reshape(128, NCH, R).transpose(1, 0, 2).reshape(DIM, R).T


def kernel(**inputs):
    global LAST_RES
    x = np.asarray(inputs["x"], np.float32)
    shared = host_prep(inputs)
    nc = _get_nc(WDT)
    cls = np.ascontiguousarray(x[:, 0, :])
    in_maps = []
    for r in range(NCORES):
        m = dict(shared)
        m["clsT"] = make_clsT(cls, r)
        in_maps.append(m)
    res = run_bass_kernel_spmd(nc, in_maps, list(range(NCORES)), trace=TRACE)
    LAST_RES = res
    out = x.copy()
    for r in range(NCORES):
        out[r * R:(r + 1) * R, 0, :] = decode_out(res.results[r]["outT"])
    return out



# revision 4
# speedup vs baseline: 2.1788x; 2.1788x over previous
# Trainium2 Bass kernel for nn_ClassBlock (mamba + EinFFT class-token block),
# tensor-parallel over 8 NeuronCores.
#
# The block only transforms x[:, :1] (the CLS token); x[:, 1:] passes through
# untouched.  Per batch row (seq len 1, h0 = 0) the math collapses to:
#   ln1  = layernorm(cls)
#   xz   = ln1 @ in_proj_w.T            -> xin, z (3072 each)
#   xc   = silu(xin * conv_w[:,3] + conv_b)
#   dbl  = xc @ x_proj_w.T              -> dt0(96), B(64), C(64)
#   dt   = softplus(dt0 @ dt_proj_w.T + dt_proj_b)
#   s    = dot(B, C)
#   y    = xc * (dt*s + D) * silu(z)
#   cls2 = cls + y @ out_proj_w.T
#   einfft(layernorm(cls2)) -> residual add
#
# Sharding: tensor parallel.  Every core holds all 64 batch rows
# (transposed activations [features(partitions), rows(free=64)]) and a
# 384-wide d_inner shard of the mamba weights; two small HBM AllReduces
# combine the x_proj and out_proj partial sums.  The EinFFT block matmuls
# are split by FFT block: core pair b=r//2 owns block b; FFT4 block
# selection and layer-2 real/imag assignment are pure per-core DATA
# (selector scalars in `sv`, sign-folded weight shards), so all 8 cores
# run one SPMD program.  The final IFFT4 (a few adds) + residual + output
# assembly happen on host, which avoids a third collective.
# Both 1/sqrt(4) ortho FFT factors are folded into the einfft weights
# host-side (softshrink lambda rescaled to l/2).

import numpy as np
from contextlib import ExitStack

import concourse.bass as bass
import concourse.mybir as mybir
import concourse.tile as tile
from concourse.bass_utils import run_bass_kernel_spmd

F32 = mybir.dt.float32
BF16 = mybir.dt.bfloat16
AF = mybir.ActivationFunctionType
ALU = mybir.AluOpType

NCORES = 8
R = 64                # batch rows per core (all of them)
DIM = 1536
NCH = DIM // 128      # 12 feature chunks
SH = 384              # d_inner shard per core
SHCH = SH // 128      # 3 shard chunks
EPS = 1e-5
LAM2 = 0.005          # softshrink lambda (0.01) folded by the 1/2 ifft factor

# smallvec column layout (per-feature vectors packed as [128, col])
SV_N1G = 0
SV_N1B = 12
SV_CW3 = 24
SV_CB = 27
SV_DTB = 30
SV_D = 33
SV_N2G = 36
SV_N2B = 48
SV_CB1R = 60
SV_CB1I = 63
SV_SS1 = 66
SV_SS2 = 69
SV_XR = 72     # FFT4 selector coeffs (4 cols each, same value all partitions)
SV_XI = 76
SV_NXI = 80
SV_TOT = 84


class _SplitDrainTC(tile.TileContext):
    """TileContext whose kernel-tail drain carries at most one sem wait.

    The neuronxcc walrus build used under axon rejects CTRL instructions
    with several sync waits ("Too many sync wait commands"), so the excess
    waits are peeled onto extra single-wait drains.
    """

    def _drain_and_barrier(self, tick_clock, wait_clock):
        from concourse.vector_clock import ScopedClock

        drain_inst = self.nc.sync.drain()
        wait_clock.add_sem_waits(
            drain_inst.ins, ScopedClock({None: tick_clock.global_clock})
        )
        si = drain_inst.ins.sync_info
        if si is not None and len(si.on_wait) > 1:
            waits = list(si.on_wait)
            drain_inst.ins.sync_info = mybir.SyncInfo(
                on_wait=[waits[0]], on_update=list(si.on_update)
            )
            for w in waits[1:]:
                d2 = self.nc.sync.drain()
                d2.ins.sync_info = mybir.SyncInfo(on_wait=[w], on_update=[])

        self.nc.all_engine_barrier()
        assert self.sems is not None
        popped = self.nc._tile_sem_poison_stack.pop()
        assert popped is self._sem_poison
        self.nc.clear_and_free_semaphores(list(self.sems.allocated().values()))
        self.nc.all_engine_barrier()


def _split_waits(nc, maxw=1):
    """Walrus (neuronxcc) allows very few sync waits per ISA instruction.

    Peel excess sem waits off every instruction onto same-engine NoOps
    inserted immediately before it -- semantically identical: the engine
    sequencer blocks on the NoOp's wait, then on the instruction's own.
    """
    for f in nc.m.functions:
        for blk in f.blocks:
            insts = list(blk.instructions)
            out = []
            changed = False
            for inst in insts:
                si = inst.sync_info
                if si is not None and len(si.on_wait) > maxw:
                    waits = list(si.on_wait)
                    for j, w in enumerate(waits[maxw:]):
                        nop = mybir.InstNoOp(
                            name=f"{inst.name}.wsp{j}", engine=inst.engine,
                            ins=[], outs=[],
                            sync_info=mybir.SyncInfo(on_wait=[w], on_update=[]),
                        )
                        out.append(nop)
                    inst.sync_info = mybir.SyncInfo(
                        on_wait=waits[:maxw], on_update=list(si.on_update)
                    )
                    changed = True
                out.append(inst)
            if changed:
                blk.instructions = out


def _layernorm_t(nc, pools, x_aps, gcol, bcol, out_dt=F32):
    """LayerNorm over features for transposed APs x_aps (12 x [128, R])."""
    psmall, tmp, sv, ones128, ones1, outp, eps_t = (
        pools["psmall"], pools["tmp"], pools["sv"], pools["ones128"],
        pools["ones1"], pools["acts"], pools["eps"],
    )
    sum_ps = psmall.tile([1, R], F32, tag="psmall")
    ssq_ps = psmall.tile([1, R], F32, tag="psmall")
    for c in range(NCH):
        sq = tmp.tile([128, R], F32, tag="tmp")
        nc.scalar.activation(sq[:], x_aps[c], AF.Square)
        nc.tensor.matmul(sum_ps[:], ones128[:], x_aps[c],
                         start=(c == 0), stop=(c == NCH - 1))
        nc.tensor.matmul(ssq_ps[:], ones128[:], sq[:],
                         start=(c == 0), stop=(c == NCH - 1))
    m_sb = tmp.tile([1, R], F32, tag="s1")
    nc.scalar.activation(m_sb[:], sum_ps[:], AF.Copy, scale=1.0 / DIM)
    ms_sb = tmp.tile([1, R], F32, tag="s1")
    nc.scalar.activation(ms_sb[:], ssq_ps[:], AF.Copy, scale=1.0 / DIM)
    mm_sb = tmp.tile([1, R], F32, tag="s1")
    nc.vector.tensor_mul(mm_sb[:], m_sb[:], m_sb[:])
    var_sb = tmp.tile([1, R], F32, tag="s1")
    nc.vector.tensor_sub(var_sb[:], ms_sb[:], mm_sb[:])
    # rstd = (var+eps)^-0.5 = exp(-0.5*ln(var+eps))
    lnv_sb = tmp.tile([1, R], F32, tag="s1")
    nc.scalar.activation(lnv_sb[:], var_sb[:], AF.Ln, bias=eps_t[:])
    rstd_sb = tmp.tile([1, R], F32, tag="s1")
    nc.scalar.activation(rstd_sb[:], lnv_sb[:], AF.Exp, scale=-0.5)
    mr_sb = tmp.tile([1, R], F32, tag="s1")
    nc.vector.tensor_mul(mr_sb[:], m_sb[:], rstd_sb[:])
    bcsrc = tmp.tile([1, 2 * R], F32, tag="s2")
    nc.vector.tensor_copy(bcsrc[:, 0:R], rstd_sb[:])
    nc.vector.tensor_copy(bcsrc[:, R:2 * R], mr_sb[:])
    bc_ps = psmall.tile([128, 2 * R], F32, tag="psmall")
    nc.tensor.matmul(bc_ps[:], ones1[:], bcsrc[:], start=True, stop=True)
    bc_sb = tmp.tile([128, 2 * R], F32, tag="s3")
    nc.scalar.activation(bc_sb[:], bc_ps[:], AF.Copy)

    outs = []
    for c in range(NCH):
        t = tmp.tile([128, R], F32, tag="tmp")
        nc.vector.tensor_mul(t[:], x_aps[c], bc_sb[:, 0:R])
        t2 = tmp.tile([128, R], F32, tag="tmp")
        nc.vector.tensor_sub(t2[:], t[:], bc_sb[:, R:2 * R])
        o = outp.tile([128, R], out_dt, tag="ln")
        nc.vector.tensor_scalar(o[:], t2[:], sv[:, gcol + c:gcol + c + 1],
                                sv[:, bcol + c:bcol + c + 1], ALU.mult, ALU.add)
        outs.append(o)
    return outs


def build_bass(wdt=F32, krep=1):
    nc = bass.Bass("TRN2", num_devices=NCORES)
    clsT_h = nc.declare_dram_parameter("clsT", [128, NCH * R], F32, isOutput=False)
    sv_h = nc.declare_dram_parameter("sv", [128, SV_TOT], F32, isOutput=False)
    w1a_h = nc.declare_dram_parameter("w1a", [128, 4608], wdt, isOutput=False)
    w1b_h = nc.declare_dram_parameter("w1b", [128, 4608], wdt, isOutput=False)
    xpw_h = nc.declare_dram_parameter("xpw", [128, 768], wdt, isOutput=False)
    dtw_h = nc.declare_dram_parameter("dtw", [96, SH], wdt, isOutput=False)
    w2_h = nc.declare_dram_parameter("w2", [128, 4608], wdt, isOutput=False)
    fw1_h = nc.declare_dram_parameter("fw1", [128, 2304], wdt, isOutput=False)
    fw2_h = nc.declare_dram_parameter("fw2", [128, 2304], wdt, isOutput=False)
    out_h = nc.declare_dram_parameter("outT", [128, (NCH + SHCH) * R], F32,
                                      isOutput=True)

    with _SplitDrainTC(nc) as tc:
        # NRT collectives desync inside hardware loops, so benchmarking
        # variants (krep > 1) unroll the body.
        for _ in range(krep):
            with ExitStack() as ctx:
                _body(ctx, tc, nc, wdt, clsT_h, sv_h, w1a_h, w1b_h, xpw_h,
                      dtw_h, w2_h, fw1_h, fw2_h, out_h[:])
    _split_waits(nc)
    return nc


def _body(ctx, tc, nc, wdt, clsT_h, sv_h, w1a_h, w1b_h, xpw_h, dtw_h, w2_h,
          fw1_h, fw2_h, out_ap):
    const = ctx.enter_context(tc.tile_pool(name="const", bufs=1))
    tmp = ctx.enter_context(tc.tile_pool(name="tmp", bufs=8))
    acts = ctx.enter_context(tc.tile_pool(name="acts", bufs=24))
    pps = ctx.enter_context(tc.tile_pool(name="pps", bufs=6, space="PSUM"))
    psmall = ctx.enter_context(tc.tile_pool(name="psmall", bufs=2, space="PSUM"))
    dram = ctx.enter_context(tc.tile_pool(name="dram", bufs=4, space="DRAM"))

    adt = wdt

    # ---- inputs: small first, then weights in order of first use
    clsT = const.tile([128, NCH * R], F32)
    nc.sync.dma_start(clsT[:], clsT_h[:])
    sv = const.tile([128, SV_TOT], F32)
    nc.sync.dma_start(sv[:], sv_h[:])
    w1a = const.tile([128, 4608], wdt)
    nc.sync.dma_start(w1a[:], w1a_h[:])
    w1b = const.tile([128, 4608], wdt)
    nc.sync.dma_start(w1b[:], w1b_h[:])
    xpw = const.tile([128, 768], wdt)
    nc.sync.dma_start(xpw[:], xpw_h[:])
    dtw = const.tile([96, SH], wdt)
    nc.sync.dma_start(dtw[:], dtw_h[:])
    w2 = const.tile([128, 4608], wdt)
    nc.sync.dma_start(w2[:], w2_h[:])
    fw1 = const.tile([128, 2304], wdt)
    nc.sync.dma_start(fw1[:], fw1_h[:])
    fw2 = const.tile([128, 2304], wdt)
    nc.sync.dma_start(fw2[:], fw2_h[:])

    ones128 = const.tile([128, 1], F32)
    nc.vector.memset(ones128[:], 1.0)
    ones1 = const.tile([1, 128], F32)
    nc.vector.memset(ones1[:], 1.0)
    eps_t = const.tile([1, 1], F32)
    nc.vector.memset(eps_t[:], EPS)

    pools = {"psmall": psmall, "tmp": tmp, "sv": sv, "ones128": ones128,
             "ones1": ones1, "acts": acts, "eps": eps_t}

    cls_aps = [clsT[:, c * R:(c + 1) * R] for c in range(NCH)]
    ln1 = _layernorm_t(nc, pools, cls_aps, SV_N1G, SV_N1B, out_dt=adt)

    # ---- in_proj shard: 6 out-chunks (3 xin -> xc via silu-conv, 3 z -> silu)
    xc = [None] * SHCH
    sz = [None] * SHCH
    for m in range(6):
        wsb = w1a if m < 3 else w1b
        mm = m if m < 3 else m - 3
        ps = pps.tile([128, R], F32, tag="ps")
        for k in range(NCH):
            off = (mm * NCH + k) * 128
            nc.tensor.matmul(ps[:], wsb[:, off:off + 128], ln1[k][:],
                             start=(k == 0), stop=(k == NCH - 1))
        t = acts.tile([128, R], adt, tag="xcz")
        if m < 3:
            nc.scalar.activation(t[:], ps[:], AF.Silu,
                                 scale=sv[:, SV_CW3 + m:SV_CW3 + m + 1],
                                 bias=sv[:, SV_CB + m:SV_CB + m + 1])
            xc[m] = t
        else:
            nc.scalar.activation(t[:], ps[:], AF.Silu)
            sz[m - 3] = t

    # ---- x_proj partials: dt0[96] (padded 128), B[64], C[64]
    ps_dt = pps.tile([128, R], F32, tag="ps")
    ps_B = pps.tile([64, R], F32, tag="ps")
    ps_C = pps.tile([64, R], F32, tag="ps")
    for k in range(SHCH):
        base = k * 256
        nc.tensor.matmul(ps_dt[:], xpw[:, base:base + 128], xc[k][:],
                         start=(k == 0), stop=(k == SHCH - 1))
        nc.tensor.matmul(ps_B[:], xpw[:, base + 128:base + 192], xc[k][:],
                         start=(k == 0), stop=(k == SHCH - 1))
        nc.tensor.matmul(ps_C[:], xpw[:, base + 192:base + 256], xc[k][:],
                         start=(k == 0), stop=(k == SHCH - 1))

    # ---- AllReduce #1: [128, 3R] = [dt0pad | B | C] partials
    ar1_sb = const.tile([128, 3 * R], F32)
    nc.vector.memset(ar1_sb[:], 0.0)
    nc.scalar.activation(ar1_sb[:, 0:R], ps_dt[:], AF.Copy)
    nc.scalar.activation(ar1_sb[0:64, R:2 * R], ps_B[:], AF.Copy)
    nc.scalar.activation(ar1_sb[0:64, 2 * R:3 * R], ps_C[:], AF.Copy)
    ar1_din = dram.tile([128, 3 * R], F32)
    ar1_dout = dram.tile([128, 3 * R], F32)
    nc.sync.dma_start(ar1_din[:], ar1_sb[:])
    nc.gpsimd.collective_compute(
        "AllReduce", ALU.add, replica_groups=[list(range(NCORES))],
        ins=[ar1_din[:].opt()], outs=[ar1_dout[:].opt()])
    ar1r = const.tile([128, 3 * R], F32)
    nc.sync.dma_start(ar1r[:], ar1_dout[:])

    # ---- s = dot(B, C) per row, broadcast to all partitions
    bcm = tmp.tile([64, R], F32, tag="tmp")
    nc.vector.tensor_mul(bcm[:], ar1r[0:64, R:2 * R], ar1r[0:64, 2 * R:3 * R])
    s_ps = psmall.tile([1, R], F32, tag="psmall")
    nc.tensor.matmul(s_ps[:], ones128[0:64, :], bcm[:], start=True, stop=True)
    s_sb = tmp.tile([1, R], F32, tag="s1")
    nc.scalar.activation(s_sb[:], s_ps[:], AF.Copy)
    sbc_ps = psmall.tile([128, R], F32, tag="psmall")
    nc.tensor.matmul(sbc_ps[:], ones1[:], s_sb[:], start=True, stop=True)
    s_bc = const.tile([128, R], F32)
    nc.scalar.activation(s_bc[:], sbc_ps[:], AF.Copy)

    # ---- dt = softplus(dtw @ dt0 + b); y = xc * (dt*s + D) * silu(z)
    dtm = const.tile([96, R], adt)
    nc.scalar.activation(dtm[:], ar1r[0:96, 0:R], AF.Copy)
    y = [None] * SHCH
    for m in range(SHCH):
        ps = pps.tile([128, R], F32, tag="ps")
        nc.tensor.matmul(ps[:], dtw[:, m * 128:(m + 1) * 128], dtm[:],
                         start=True, stop=True)
        e = tmp.tile([128, R], F32, tag="tmp")
        nc.scalar.activation(e[:], ps[:], AF.Exp,
                             bias=sv[:, SV_DTB + m:SV_DTB + m + 1])
        dt_t = tmp.tile([128, R], F32, tag="tmp")
        nc.scalar.activation(dt_t[:], e[:], AF.Ln, bias=ones128[:, 0:1])
        dts = tmp.tile([128, R], F32, tag="tmp")
        nc.vector.tensor_mul(dts[:], dt_t[:], s_bc[:])
        xsz = tmp.tile([128, R], F32, tag="tmp")
        nc.vector.tensor_mul(xsz[:], xc[m][:], sz[m][:])
        yt = acts.tile([128, R], adt, tag="y")
        nc.vector.scalar_tensor_tensor(yt[:], dts[:], sv[:, SV_D + m:SV_D + m + 1],
                                       xsz[:], ALU.add, ALU.mult)
        y[m] = yt

    # ---- out_proj partials -> AllReduce #2
    ar2_sb = const.tile([128, NCH * R], F32)
    for m in range(NCH):
        ps = pps.tile([128, R], F32, tag="ps")
        for k in range(SHCH):
            off = (m * SHCH + k) * 128
            nc.tensor.matmul(ps[:], w2[:, off:off + 128], y[k][:],
                             start=(k == 0), stop=(k == SHCH - 1))
        nc.scalar.activation(ar2_sb[:, m * R:(m + 1) * R], ps[:], AF.Copy)
    ar2_din = dram.tile([128, NCH * R], F32)
    ar2_dout = dram.tile([128, NCH * R], F32)
    nc.sync.dma_start(ar2_din[:], ar2_sb[:])
    nc.gpsimd.collective_compute(
        "AllReduce", ALU.add, replica_groups=[list(range(NCORES))],
        ins=[ar2_din[:].opt()], outs=[ar2_dout[:].opt()])
    ar2r = const.tile([128, NCH * R], F32)
    nc.sync.dma_start(ar2r[:], ar2_dout[:])

    # ---- cls2 = cls + mam (also the first 768 cols of the output slab)
    out_sb = const.tile([128, (NCH + SHCH) * R], F32)
    cls2_aps = []
    for c in range(NCH):
        ap = out_sb[:, c * R:(c + 1) * R]
        nc.vector.tensor_add(ap, ar2r[:, c * R:(c + 1) * R],
                             clsT[:, c * R:(c + 1) * R])
        cls2_aps.append(ap)

    ln2 = _layernorm_t(nc, pools, cls2_aps, SV_N2G, SV_N2B, out_dt=F32)

    # ---- FFT4 block selection via per-core sv coeffs: 4-term weighted sums
    def fft_sel(base, cc, dst_dt):
        t0 = tmp.tile([128, R], F32, tag="tmp")
        nc.vector.tensor_scalar_mul(t0[:], ln2[cc][:], sv[:, base:base + 1])
        t1 = tmp.tile([128, R], F32, tag="tmp")
        nc.vector.scalar_tensor_tensor(t1[:], ln2[3 + cc][:],
                                       sv[:, base + 1:base + 2], t0[:],
                                       ALU.mult, ALU.add)
        t2 = tmp.tile([128, R], F32, tag="tmp")
        nc.vector.scalar_tensor_tensor(t2[:], ln2[6 + cc][:],
                                       sv[:, base + 2:base + 3], t1[:],
                                       ALU.mult, ALU.add)
        o = acts.tile([128, R], dst_dt, tag="fft")
        nc.vector.scalar_tensor_tensor(o[:], ln2[9 + cc][:],
                                       sv[:, base + 3:base + 4], t2[:],
                                       ALU.mult, ALU.add)
        return o

    XR = [fft_sel(SV_XR, cc, adt) for cc in range(3)]
    XI = [fft_sel(SV_XI, cc, adt) for cc in range(3)]
    NXI = [fft_sel(SV_NXI, cc, adt) for cc in range(3)]

    # ---- einfft layer 1 (this core's block): r1 = relu(W10.XR + W11.NXI + br)
    #                                          i1n = -relu(W11.XR + W10.XI + bi)
    r1 = [None] * 3
    i1n = [None] * 3
    for mc in range(3):
        ps_r = pps.tile([128, R], F32, tag="ps")
        ps_i = pps.tile([128, R], F32, tag="ps")
        for kc in range(3):
            c0 = kc * SH + mc * 128
            nc.tensor.matmul(ps_r[:], fw1[:, c0:c0 + 128], XR[kc][:],
                             start=(kc == 0), stop=False)
            nc.tensor.matmul(ps_i[:], fw1[:, 1152 + c0:1152 + c0 + 128],
                             XR[kc][:], start=(kc == 0), stop=False)
        for kc in range(3):
            c0 = kc * SH + mc * 128
            nc.tensor.matmul(ps_r[:], fw1[:, 1152 + c0:1152 + c0 + 128],
                             NXI[kc][:], start=False, stop=(kc == 2))
            nc.tensor.matmul(ps_i[:], fw1[:, c0:c0 + 128], XI[kc][:],
                             start=False, stop=(kc == 2))
        rt = acts.tile([128, R], adt, tag="r1")
        nc.scalar.activation(rt[:], ps_r[:], AF.Relu,
                             bias=sv[:, SV_CB1R + mc:SV_CB1R + mc + 1])
        it = tmp.tile([128, R], F32, tag="tmp")
        nc.scalar.activation(it[:], ps_i[:], AF.Relu,
                             bias=sv[:, SV_CB1I + mc:SV_CB1I + mc + 1])
        nt = acts.tile([128, R], adt, tag="i1n")
        nc.vector.tensor_scalar_mul(nt[:], it[:], -1.0)
        r1[mc] = rt
        i1n[mc] = nt

    # ---- einfft layer 2 + softshrink: OUT = ss(W2a.r1 + W2b.i1n + cb2')
    for mc in range(3):
        ps = pps.tile([128, R], F32, tag="ps")
        for kc in range(3):
            c0 = kc * SH + mc * 128
            nc.tensor.matmul(ps[:], fw2[:, c0:c0 + 128], r1[kc][:],
                             start=(kc == 0), stop=False)
        for kc in range(3):
            c0 = kc * SH + mc * 128
            nc.tensor.matmul(ps[:], fw2[:, 1152 + c0:1152 + c0 + 128],
                             i1n[kc][:], start=False, stop=(kc == 2))
        a1 = tmp.tile([128, R], F32, tag="tmp")
        nc.scalar.activation(a1[:], ps[:], AF.Relu,
                             bias=sv[:, SV_SS1 + mc:SV_SS1 + mc + 1])
        a2 = tmp.tile([128, R], F32, tag="tmp")
        nc.scalar.activation(a2[:], ps[:], AF.Relu, scale=-1.0,
                             bias=sv[:, SV_SS2 + mc:SV_SS2 + mc + 1])
        nc.vector.tensor_sub(out_sb[:, (NCH + mc) * R:(NCH + mc + 1) * R],
                             a1[:], a2[:])

    nc.sync.dma_start(out_ap, out_sb[:])


# ---------------------------------------------------------------------------
# Host side
# ---------------------------------------------------------------------------

_NC_CACHE = {}
WDT = BF16  # weight dtype knob (bf16 halves the weight-stream DMA)


def _np_wdt(wdt):
    if wdt == F32:
        return np.float32
    import ml_dtypes
    return ml_dtypes.bfloat16


def _get_nc(wdt):
    if wdt not in _NC_CACHE:
        _NC_CACHE[wdt] = build_bass(wdt)
    return _NC_CACHE[wdt]


def _chunkcols(v):
    """[C*128] feature vector -> [128, C] (feature f=128c+p at [p, c])."""
    v = np.asarray(v, np.float32)
    C = v.shape[0] // 128
    return v.reshape(C, 128).T


def _pack_lhsT(W, kch, mch):
    """W [kch*128, mch*128] -> [128, kch? ...] m-major is NOT used here:
    arr[p, kc*mch*128 + mc*128 + c] = W[kc*128+p, mc*128+c]."""
    return np.ascontiguousarray(
        W.reshape(kch, 128, mch, 128).transpose(1, 0, 2, 3)
        .reshape(128, kch * mch * 128))


def host_prep(inputs, wdt=None):
    """Build the 8 per-core device input maps."""
    wdt = wdt or WDT
    nw = _np_wdt(wdt)
    g = lambda k: np.asarray(inputs[k], np.float32)

    x = g("x")
    cls = np.ascontiguousarray(x[:, 0, :])       # [64, 1536]
    clsT = np.ascontiguousarray(
        cls.T.reshape(NCH, 128, R).transpose(1, 0, 2).reshape(128, NCH * R))

    A = g("in_proj_w")                           # [6144, 1536]
    XP = g("x_proj_w")                           # [224, 3072]
    DTW = g("dt_proj_w")                         # [3072, 96]
    O = g("out_proj_w")                          # [1536, 3072]
    cw1 = g("cw1")                               # [2, 4, 384, 384]
    cw2 = g("cw2")
    cb1 = g("cb1")                               # [2, 4, 384]
    cb2 = g("cb2")

    # FFT4 selector coefficients per block (xr / xi of X_b from x_0..x_3)
    xr_coef = {0: (1, 1, 1, 1), 1: (1, 0, -1, 0),
               2: (1, -1, 1, -1), 3: (1, 0, -1, 0)}
    xi_coef = {0: (0, 0, 0, 0), 1: (0, -1, 0, 1),
               2: (0, 0, 0, 0), 3: (0, 1, 0, -1)}

    in_maps = []
    for r in range(NCORES):
        sh = slice(r * SH, (r + 1) * SH)
        b = r // 2
        h = r % 2
        # I2 only exists for odd blocks; its owner is the h=1 core there.
        ri = 1 if (b % 2 == 1 and h == 1) else 0

        W1 = np.concatenate([A[sh], A[3072 + r * SH:3072 + (r + 1) * SH]])
        W1T = W1.T                                # [1536, 768]
        w1 = np.ascontiguousarray(
            W1T.reshape(12, 128, 6, 128).transpose(1, 2, 0, 3)
            .reshape(128, 9216)).astype(nw)       # m-major [m][k]

        XPT = XP[:, sh].T                         # [384, 224]
        XTpad = np.zeros((SH, 256), np.float32)
        XTpad[:, 0:96] = XPT[:, 0:96]
        XTpad[:, 128:192] = XPT[:, 96:160]
        XTpad[:, 192:256] = XPT[:, 160:224]
        xpw = np.ascontiguousarray(
            XTpad.reshape(3, 128, 256).transpose(1, 0, 2)
            .reshape(128, 768)).astype(nw)

        dtw = np.ascontiguousarray(DTW[sh].T).astype(nw)   # [96, 384]

        OT = O[:, sh].T                           # [384, 1536]
        w2 = np.ascontiguousarray(
            OT.reshape(3, 128, 12, 128).transpose(1, 2, 0, 3)
            .reshape(128, 4608)).astype(nw)       # m-major [m][k]

        W10 = 0.5 * cw1[0, b]
        W11 = 0.5 * cw1[1, b]
        fw1 = np.concatenate([_pack_lhsT(W10, 3, 3), _pack_lhsT(W11, 3, 3)],
                             axis=1).astype(nw)
        W20 = 0.5 * cw2[0, b]
        W21 = 0.5 * cw2[1, b]
        if ri == 0:
            W2a, W2b = W20, W21
        else:
            W2a, W2b = W21, -W20
        fw2 = np.concatenate([_pack_lhsT(W2a, 3, 3), _pack_lhsT(W2b, 3, 3)],
                             axis=1).astype(nw)

        sv = np.zeros((128, SV_TOT), np.float32)
        sv[:, SV_N1G:SV_N1G + 12] = _chunkcols(g("norm1_g"))
        sv[:, SV_N1B:SV_N1B + 12] = _chunkcols(g("norm1_b"))
        sv[:, SV_CW3:SV_CW3 + 3] = _chunkcols(g("conv_w")[sh, 3])
        sv[:, SV_CB:SV_CB + 3] = _chunkcols(g("conv_b")[sh])
        sv[:, SV_DTB:SV_DTB + 3] = _chunkcols(g("dt_proj_b")[sh])
        sv[:, SV_D:SV_D + 3] = _chunkcols(g("D")[sh])
        sv[:, SV_N2G:SV_N2G + 12] = _chunkcols(g("norm2_g"))
        sv[:, SV_N2B:SV_N2B + 12] = _chunkcols(g("norm2_b"))
        sv[:, SV_CB1R:SV_CB1R + 3] = _chunkcols(cb1[0, b])
        sv[:, SV_CB1I:SV_CB1I + 3] = _chunkcols(cb1[1, b])
        sv[:, SV_SS1:SV_SS1 + 3] = _chunkcols(0.5 * cb2[ri, b] - LAM2)
        sv[:, SV_SS2:SV_SS2 + 3] = _chunkcols(-0.5 * cb2[ri, b] - LAM2)
        for j in range(4):
            sv[:, SV_XR + j] = xr_coef[b][j]
            sv[:, SV_XI + j] = xi_coef[b][j]
            sv[:, SV_NXI + j] = -xi_coef[b][j]

        in_maps.append({
            "clsT": clsT, "sv": sv,
            "w1a": np.ascontiguousarray(w1[:, :4608]),
            "w1b": np.ascontiguousarray(w1[:, 4608:]),
            "xpw": xpw, "dtw": dtw, "w2": w2, "fw1": fw1, "fw2": fw2,
        })
    return in_maps


def _dec12(a):
    """[128, 12*64] transposed chunks -> [64 rows, 1536 feats]."""
    return np.asarray(a, np.float32).reshape(128, NCH, R).transpose(1, 0, 2) \
        .reshape(DIM, R).T


def _dec3(a):
    """[128, 3*64] transposed chunks -> [384 feats, 64 rows]."""
    return np.asarray(a, np.float32).reshape(128, 3, R).transpose(1, 0, 2) \
        .reshape(SH, R)


def kernel(**inputs):
    x = np.asarray(inputs["x"], np.float32)
    in_maps = host_prep(inputs)
    nc = _get_nc(WDT)
    res = run_bass_kernel_spmd(nc, in_maps, list(range(NCORES)))
    outs = [np.asarray(res.results[r]["outT"], np.float32)
            for r in range(NCORES)]
    cls2 = _dec12(outs[0][:, :NCH * R])                   # [64, 1536]
    piece = [_dec3(o[:, NCH * R:]) for o in outs]         # [384, 64] each
    R0, R1, I1 = piece[0], piece[2], piece[3]
    R2_, R3, I3 = piece[4], piece[6], piece[7]
    a = R0 + R2_
    bb = R1 + R3
    c = R0 - R2_
    d = I1 - I3
    y = np.concatenate([a + bb, c - d, a - bb, c + d], axis=0)  # [1536, 64]
    out = x.copy()
    out[:, 0, :] = cls2 + y.T
    return out
